# revision 1
# baseline (speedup 1.0000x reference)
"""Trainium2 Bass kernel for nn_DASAttentionGate (depthwise-sep conv -> InstanceNorm
-> ReLU -> offset conv -> deformable conv -> GroupNorm -> sigmoid gate).

Sharding: 8 cores = 4 samples x 2 H-halves (48 output rows each). Cross-core
communication: two tiny AllReduces (InstanceNorm + GroupNorm statistics) within
sample pairs.

Deformable conv strategy ("quad gather"):
  - h_n (normalized activations) transposed to pixel-major h_T and written to a
    DRAM table z4 of 2x2 pixel quads: z4[(y,x)] = [h(y,x), h(y,x+1), h(y+1,x),
    h(y+1,x+1)] for all 128 channels (bf16, 1KB per block).
  - Offsets -> per (tap, pixel) a single int16 block index + 4 bilinear corner
    weights (device-computed, staged through DRAM to re-tile into the gather's
    pixel-mod-128 layout).
  - gpsimd.dma_gather fetches one 1KB quad per (tap, pixel).
  - DVE scales the 4 corners by their weights (bf16 2x mode via duplicated-mask
    APs), then PE sums corners + transposes back to channel-major via 4
    accumulating identity-RHS matmuls, and finally contracts channels with the
    deform weights (einsum), accumulating all 9 taps in PSUM.
"""

import os
import sys

for _p in ("/opt/trn_rl_repo",):
    if os.path.isdir(_p) and _p not in sys.path:
        sys.path.insert(0, _p)

import numpy as np
import ml_dtypes

import concourse.bass as bass
import concourse.bacc as bacc
import concourse.tile as tile
from concourse import mybir
from concourse.bass_utils import run_bass_kernel_spmd

F32 = mybir.dt.float32
F32R = mybir.dt.float32r
BF16 = mybir.dt.bfloat16
I16 = mybir.dt.int16
I32 = mybir.dt.int32
AF = mybir.ActivationFunctionType
OP = mybir.AluOpType

# problem geometry (hardcoded per the task contract)
B, C, H, W = 4, 128, 96, 96
NCORES = 8
RPC = 48           # output rows per core
GR, GC = 64, 112   # h grid: rows r0-8..r0+55, cols -8..103
XR, XC = 66, 114   # x grid: rows r0-9..r0+56, cols -9..104
ZY, ZX = 63, 112   # quad block grid: y0' in 0..62, x0' in 0..110 (stride ZX)
NB = ZY * ZX       # 7056 blocks
NPIX = RPC * W     # 4608 output pixels per core
NT = NPIX // 128   # 36 pixel tiles
BLKT = 9           # pixel tiles per gather block
NBLK = NT // BLKT  # 3 gather blocks
NIDX = BLKT * 128  # 1536 indices per gather call
EPS = 1e-5

_CACHE = {}


def _build_program():
    nc = bacc.Bacc("TRN2", target_bir_lowering=False, debug=False,
                   num_devices=NCORES)

    # ---- I/O ----
    x_d = nc.dram_tensor("x_sh", [C, XR, XC], F32, kind="ExternalInput")
    vm_d = nc.dram_tensor("vrow", [C, GR], BF16, kind="ExternalInput")
    wf_d = nc.dram_tensor("wf", [C, 9, C], F32, kind="ExternalInput")
    b1_d = nc.dram_tensor("b1c", [C, 1], F32, kind="ExternalInput")
    ow_d = nc.dram_tensor("ow", [C, 9, 18], BF16, kind="ExternalInput")
    ob_d = nc.dram_tensor("obr", [128, 18], F32, kind="ExternalInput")
    wd_d = nc.dram_tensor("wd", [C, 9, C], BF16, kind="ExternalInput")
    db_d = nc.dram_tensor("dbc", [C, 1], F32, kind="ExternalInput")
    gw_d = nc.dram_tensor("gwc", [C, 1], F32, kind="ExternalInput")
    gb_d = nc.dram_tensor("gbc", [C, 1], F32, kind="ExternalInput")
    id_d = nc.dram_tensor("idn", [128, 128], BF16, kind="ExternalInput")
    on_d = nc.dram_tensor("onec", [C, 1], F32, kind="ExternalInput")
    io_d = nc.dram_tensor("iotc", [96, RPC, 9], F32, kind="ExternalInput")
    out_d = nc.dram_tensor("out_sh", [C, RPC, W], F32, kind="ExternalOutput")

    groups = [[0, 1], [2, 3], [4, 5], [6, 7]]

    with tile.TileContext(nc) as tc:
        with (
            tc.tile_pool(name="const", bufs=1) as constp,
            tc.tile_pool(name="xbuf", bufs=1) as xpool,
            tc.tile_pool(name="hbuf", bufs=1) as hpool,
            tc.tile_pool(name="mwork", bufs=1) as mpool,
            tc.tile_pool(name="gbuf", bufs=1) as gpool,
            tc.tile_pool(name="sbig", bufs=1) as spool,
            tc.tile_pool(name="ps", bufs=6, space="PSUM") as psp,
            tc.tile_pool(name="dram", bufs=1, space="DRAM") as dramp,
        ):
            # ---- load constants ----
            wf = constp.tile([C, 9, C], F32R, tag="wf")
            nc.sync.dma_start(wf[:], wf_d[:].bitcast(F32R))
            b1 = constp.tile([C, 1], F32, tag="b1")
            nc.sync.dma_start(b1[:], b1_d[:])
            ow = constp.tile([C, 9, 18], BF16, tag="ow")
            nc.sync.dma_start(ow[:], ow_d[:])
            ob = constp.tile([128, 18], F32, tag="ob")
            nc.sync.dma_start(ob[:], ob_d[:])
            wd = constp.tile([C, 9, C], BF16, tag="wd")
            nc.sync.dma_start(wd[:], wd_d[:])
            db = constp.tile([C, 1], F32, tag="db")
            nc.sync.dma_start(db[:], db_d[:])
            gw = constp.tile([C, 1], F32, tag="gw")
            nc.sync.dma_start(gw[:], gw_d[:])
            gb = constp.tile([C, 1], F32, tag="gb")
            nc.sync.dma_start(gb[:], gb_d[:])
            idn = constp.tile([128, 128], BF16, tag="idn")
            nc.sync.dma_start(idn[:], id_d[:])
            onec = constp.tile([C, 1], F32, tag="onec")
            nc.sync.dma_start(onec[:], on_d[:])
            vm = constp.tile([C, GR], BF16, tag="vm")
            nc.sync.dma_start(vm[:], vm_d[:])

            # ---- conv1 (fused depthwise+pointwise, f32r) ----
            xs = xpool.tile([C, XR, XC], F32R, tag="xs")
            nc.sync.dma_start(xs[:, 0:33, :], x_d[:, 0:33, :].bitcast(F32R))
            nc.sync.dma_start(xs[:, 33:66, :], x_d[:, 33:66, :].bitcast(F32R))

            hraw = hpool.tile([C, GR, GC], F32, tag="hraw")
            CH = 4  # grid rows per psum chunk
            for ch in range(GR // CH):
                gr0 = ch * CH
                pt = psp.tile([128, CH * GC], F32, tag="ps")
                for t in range(9):
                    ty, tx = t // 3, t % 3
                    rhs = xs[:, gr0 + ty:gr0 + ty + CH, tx:tx + GC]
                    nc.tensor.matmul(
                        pt[:], wf[:, t, :], rhs,
                        start=(t == 0), stop=(t == 8))
                nc.scalar.activation(
                    hraw[:, gr0:gr0 + CH, :].rearrange("p a b -> p (a b)"),
                    pt[:], AF.Identity, bias=b1[:])

            # ---- InstanceNorm stats over own 48 valid rows ----
            valid = hraw[:, 8:56, 8:104]
            st = mpool.tile([C, 2], F32, tag="st")
            nc.vector.tensor_reduce(st[:, 0:1], valid, mybir.AxisListType.XY,
                                    OP.add)
            sq = spool.tile([C, NPIX], F32, tag="big")
            nc.scalar.activation(sq[:].rearrange("p (a b) -> p a b", a=RPC),
                                 valid, AF.Square, accum_out=st[:, 1:2])

            cc_in = dramp.tile([C, 2], F32, tag="cci")
            cc_out = dramp.tile([C, 2], F32, tag="cco")
            nc.sync.dma_start(cc_in[:], st[:])
            nc.gpsimd.collective_compute(
                "AllReduce", OP.add, replica_groups=groups,
                ins=[cc_in[:].opt()], outs=[cc_out[:].opt()])
            stg = mpool.tile([C, 2], F32, tag="stg")
            nc.sync.dma_start(stg[:], cc_out[:])

            # mean/rstd per channel
            mom = mpool.tile([C, 2], F32, tag="mom")
            nc.vector.tensor_scalar(mom[:], stg[:], 1.0 / (H * W), None,
                                    OP.mult)
            var = mpool.tile([C, 1], F32, tag="var")
            nc.vector.tensor_tensor(var[:], mom[:, 0:1], mom[:, 0:1], OP.mult)
            nc.vector.tensor_tensor(var[:], mom[:, 1:2], var[:], OP.subtract)
            nc.vector.tensor_scalar(var[:], var[:], EPS, None, OP.add)
            rstd = mpool.tile([C, 1], F32, tag="rstd")
            nc.scalar.activation(rstd[:], var[:], AF.Sqrt)
            nc.vector.reciprocal(rstd[:], rstd[:])
            nbias = mpool.tile([C, 1], F32, tag="nbias")
            nc.vector.tensor_tensor(nbias[:], mom[:, 0:1], rstd[:], OP.mult)
            nc.vector.tensor_scalar(nbias[:], nbias[:], -1.0, None, OP.mult)

            # ---- h_n (bf16, masked) + f32 shortcut ----
            hn = hpool.tile([C, GR, GC], BF16, tag="hn")
            nc.scalar.activation(hn[:], hraw[:], AF.Relu, bias=nbias[:],
                                 scale=rstd[:])
            vmb = vm[:].unsqueeze(2).broadcast_to((C, GR, GC))
            nc.vector.tensor_tensor(hn[:], hn[:], vmb, OP.mult)
            nc.gpsimd.memset(hn[:, :, 0:8], 0.0)
            nc.gpsimd.memset(hn[:, :, 104:112], 0.0)
            short = spool.tile([C, NPIX], F32, tag="short")
            nc.scalar.activation(
                short[:].rearrange("p (a b) -> p a b", a=RPC),
                hraw[:, 8:56, 8:104], AF.Relu, bias=nbias[:], scale=rstd[:])

            # ---- h_T (pixel-major transpose of h_n rows) ----
            hT = hpool.tile([112, GR, 128], BF16, tag="hraw")
            for gr in range(GR):
                pt = psp.tile([112, 128], F32, tag="ps")
                nc.tensor.matmul(pt[:], hn[:, gr, :], idn[:],
                                 start=True, stop=True)
                nc.scalar.activation(hT[:, gr, :], pt[:], AF.Copy)

            # ---- z4 quad table in DRAM ----
            z4 = dramp.tile([NB, 512], BF16, tag="z4")
            z4v = z4[:].rearrange("(y x) (j c) -> x y j c", x=ZX, j=4)
            for j, (jy, jx) in enumerate(((0, 0), (0, 1), (1, 0), (1, 1))):
                nc.sync.dma_start(z4v[0:111, :, j, :],
                                  hT[jx:jx + 111, jy:jy + ZY, :])

            # ---- offset conv (bf16, output transposed per row) ----
            offT = mpool.tile([96, RPC, 18], F32, tag="offT")
            for r in range(RPC):
                gr = r + 8
                po = psp.tile([96, 18], F32, tag="ps")
                for t in range(9):
                    ty, tx = t // 3, t % 3
                    lhsT = hn[:, gr + ty - 1, 7 + tx:7 + tx + 96]
                    nc.tensor.matmul(po[:], lhsT, ow[:, t, :],
                                     start=(t == 0), stop=(t == 8))
                nc.scalar.activation(offT[:, r, :], po[:], AF.Copy)
            obv = ob[0:96, :].unsqueeze(1).broadcast_to((96, RPC, 18))
            nc.vector.tensor_tensor(offT[:], offT[:], obv, OP.add)

            # ---- bilinear masks + gather indices ----
            # all [96, RPC, 9] f32 grids (partition = w)
            def mk(tag):
                return mpool.tile([96, RPC, 9], F32, tag=tag, name=tag)

            offv = offT[:].rearrange("p r (k two) -> p r k two", two=2)
            oy, ox = offv[:, :, :, 0], offv[:, :, :, 1]
            it32 = mpool.tile([96, RPC, 9], I32, tag="it32")
            kf = mk("kf")
            gt = mk("gt")
            fy = mk("fy")
            ly = mk("ly")
            fx = mk("fx")
            lx = mk("lx")
            for (o_, f_, l_) in ((oy, fy, ly), (ox, fx, lx)):
                nc.vector.tensor_copy(it32[:], o_)
                nc.vector.tensor_copy(kf[:], it32[:])
                nc.vector.tensor_tensor(gt[:], kf[:], o_, OP.is_gt)
                nc.vector.tensor_tensor(f_[:], kf[:], gt[:], OP.subtract)
                nc.vector.tensor_tensor(l_[:], o_, f_[:], OP.subtract)
            uy = mk("uy")
            ux = mk("ux")
            nc.vector.tensor_scalar(uy[:], ly[:], -1.0, 1.0, OP.mult, OP.add)
            nc.vector.tensor_scalar(ux[:], lx[:], -1.0, 1.0, OP.mult, OP.add)
            a_sb = mpool.tile([96, RPC, 9, 4, 2], BF16, tag="a_sb")

            def dup2(ap):
                return ap.unsqueeze(3).broadcast_to((96, RPC, 9, 2))

            for j, (fa, fb) in enumerate(((uy, ux), (uy, lx),
                                          (ly, ux), (ly, lx))):
                nc.vector.tensor_tensor(a_sb[:, :, :, j, :],
                                        dup2(fa[:]), dup2(fb[:]), OP.mult)

            # idx = iota + 112*fy + fx, clamped to [0, NB-1]
            iot = mpool.tile([96, RPC, 9], F32, tag="iot")
            nc.sync.dma_start(iot[:], io_d[:])
            idxf = mk("idxf")
            nc.vector.tensor_scalar(idxf[:], fy[:], float(ZX), None, OP.mult)
            nc.vector.tensor_tensor(idxf[:], idxf[:], fx[:], OP.add)
            nc.vector.tensor_tensor(idxf[:], idxf[:], iot[:], OP.add)
            nc.vector.tensor_scalar(idxf[:], idxf[:], 0.0, float(NB - 1),
                                    OP.max, OP.min)
            idx_sb = mpool.tile([96, RPC, 9], I16, tag="idx_sb")
            nc.vector.tensor_copy(idx_sb[:], idxf[:])

            # ---- stage masks/indices through DRAM to re-tile ----
            a_d = dramp.tile([9, NPIX, 8], BF16, tag="a_d")
            for kk in range(9):
                nc.sync.dma_start(
                    a_d[kk].rearrange("(r w) j -> w r j", w=96),
                    a_sb[:, :, kk, :, :].rearrange("p r j t -> p r (j t)"))
            idx_d = dramp.tile([9, NPIX], I16, tag="idx_d")
            for kk in range(9):
                nc.sync.dma_start(
                    idx_d[kk].rearrange("(r w) -> w r", w=96),
                    idx_sb[:, :, kk])

            # wrapped-16 gather index tiles: [128, 27 (kk,blk), NIDX/16]
            NC16 = NIDX // 16
            idx_w = gpool.tile([128, 9 * NBLK, NC16], I16, tag="idx_w")
            for kk in range(9):
                for blki in range(NBLK):
                    isrc = idx_d[kk][blki * NIDX:(blki + 1) * NIDX]
                    nc.sync.dma_start(
                        idx_w[0:16, kk * NBLK + blki, :],
                        isrc.rearrange("(col p) -> p col", p=16))
            for g16 in range(1, 8):
                nc.sync.dma_start(idx_w[g16 * 16:(g16 + 1) * 16, :, :],
                                  idx_w[0:16, :, :])
            # corner weights, duplicated pairs: [128, 9, NT, 4, 2]
            a_w = gpool.tile([128, 9, NT, 4, 2], BF16, tag="a_w")
            for kk in range(9):
                nc.sync.dma_start(
                    a_w[:, kk, :, :, :].rearrange("p g j t -> p g (j t)"),
                    a_d[kk].rearrange("(g p) j -> p g j", p=128))

            # ---- gather + scale + corner-sum/transpose + einsum ----
            d_sb = spool.tile([C, NT, 128], F32, tag="dsb")
            for blk in range(NBLK):
                sampT = xpool.tile([128, 9, BLKT, 128], BF16, tag="xs")
                for kk in range(9):
                    g_t = gpool.tile([128, BLKT, 4, 128], BF16, tag="g_t", bufs=2)
                    nc.gpsimd.dma_gather(
                        g_t[:].rearrange("p a b c -> p a (b c)"),
                        z4[:], idx_w[:, kk * NBLK + blk, :],
                        NIDX, NIDX, 512, queue_num=0,
                        single_packet=False)
                    # scale corners by bilinear weights (bf16 2x via dup pairs)
                    gv = g_t[:].rearrange("p a b (c two) -> p a b c two",
                                          two=2)
                    for j in range(4):
                        av = a_w[:, kk, blk * BLKT:(blk + 1) * BLKT, j, :]
                        av = av.unsqueeze(2).broadcast_to((128, BLKT, 64, 2))
                        nc.vector.tensor_tensor(gv[:, :, j], gv[:, :, j], av,
                                                OP.mult)
                    # sum 4 corners + transpose to channel-major via PE
                    for t in range(BLKT):
                        pt = psp.tile([128, 128], F32, tag="ps")
                        for j in range(4):
                            nc.tensor.matmul(pt[:], g_t[:, t, j, :], idn[:],
                                             start=(j == 0), stop=(j == 3))
                        nc.scalar.activation(sampT[:, kk, t, :], pt[:],
                                             AF.Copy)
                # einsum: accumulate 9 taps
                for t in range(BLKT):
                    pd = psp.tile([128, 128], F32, tag="ps")
                    for kk in range(9):
                        nc.tensor.matmul(pd[:], wd[:, kk, :], sampT[:, kk, t, :],
                                         start=(kk == 0), stop=(kk == 8))
                    nc.scalar.activation(d_sb[:, blk * BLKT + t, :], pd[:],
                                         AF.Identity, bias=db[:])

            # ---- GroupNorm stats (whole sample) ----
            gst = mpool.tile([C, 2], F32, tag="gst")
            nc.vector.tensor_reduce(gst[:, 0:1], d_sb[:],
                                    mybir.AxisListType.XY, OP.add)
            nc.scalar.activation(sq[:].rearrange("p (a b) -> p a b", a=NT),
                                 d_sb[:], AF.Square, accum_out=gst[:, 1:2])
            pg = psp.tile([1, 2], F32, tag="ps")
            nc.tensor.matmul(pg[:], onec[:], gst[:], start=True, stop=True)
            gred = mpool.tile([1, 2], F32, tag="gred")
            nc.scalar.activation(gred[:], pg[:], AF.Copy)
            ccg_in = dramp.tile([1, 2], F32, tag="ccgi")
            ccg_out = dramp.tile([1, 2], F32, tag="ccgo")
            nc.sync.dma_start(ccg_in[:], gred[:])
            nc.gpsimd.collective_compute(
                "AllReduce", OP.add, replica_groups=groups,
                ins=[ccg_in[:].opt()], outs=[ccg_out[:].opt()])
            gsc = mpool.tile([1, 2], F32, tag="gsc")
            nc.sync.dma_start(gsc[:], ccg_out[:])
            gall = mpool.tile([128, 2], F32, tag="gall")
            nc.gpsimd.partition_broadcast(gall[:], gsc[:], 128)

            gmom = mpool.tile([C, 2], F32, tag="gmom")
            nc.vector.tensor_scalar(gmom[:], gall[:], 1.0 / (C * H * W), None,
                                    OP.mult)
            gvar = mpool.tile([C, 1], F32, tag="gvar")
            nc.vector.tensor_tensor(gvar[:], gmom[:, 0:1], gmom[:, 0:1],
                                    OP.mult)
            nc.vector.tensor_tensor(gvar[:], gmom[:, 1:2], gvar[:],
                                    OP.subtract)
            nc.vector.tensor_scalar(gvar[:], gvar[:], EPS, None, OP.add)
            grstd = mpool.tile([C, 1], F32, tag="grstd")
            nc.scalar.activation(grstd[:], gvar[:], AF.Sqrt)
            nc.vector.reciprocal(grstd[:], grstd[:])
            # scale2 = gn_w * rstd ; bias2 = gn_b - mean * scale2
            sc2 = mpool.tile([C, 1], F32, tag="sc2")
            nc.vector.tensor_tensor(sc2[:], gw[:], grstd[:], OP.mult)
            bi2 = mpool.tile([C, 1], F32, tag="bi2")
            nc.vector.tensor_tensor(bi2[:], gmom[:, 0:1], sc2[:], OP.mult)
            nc.vector.tensor_tensor(bi2[:], gb[:], bi2[:], OP.subtract)

            # ---- gate + residual ----
            gg = spool.tile([C, NPIX], F32, tag="big")  # reuse sq slot
            nc.scalar.activation(gg[:].rearrange("p (a b) -> p a b", a=NT),
                                 d_sb[:], AF.Sigmoid, bias=bi2[:],
                                 scale=sc2[:])
            nc.vector.tensor_scalar(gg[:], gg[:], 1.0, None, OP.add)
            nc.vector.tensor_tensor(gg[:], gg[:], short[:], OP.mult)
            nc.sync.dma_start(
                out_d[:], gg[:].rearrange("p (r w) -> p r w", w=W))

    nc.compile()
    return nc


def _prep_inputs(inputs):
    x = np.asarray(inputs["x"], np.float32)
    dw_w = np.asarray(inputs["dw_w"], np.float32)
    dw_b = np.asarray(inputs["dw_b"], np.float32)
    pw_w = np.asarray(inputs["pw_w"], np.float32)
    pw_b = np.asarray(inputs["pw_b"], np.float32)
    off_w = np.asarray(inputs["off_w"], np.float32)
    off_b = np.asarray(inputs["off_b"], np.float32)
    de_w = np.asarray(inputs["de_w"], np.float32)
    de_b = np.asarray(inputs["de_b"], np.float32)
    gn_w = np.asarray(inputs["gn_w"], np.float32)
    gn_b = np.asarray(inputs["gn_b"], np.float32)

    bf = ml_dtypes.bfloat16
    # fused conv1 weights: wf[t][c, o] = pw_w[o, c] * dw_w[c, 0, ty, tx]
    dwt = dw_w.reshape(C, 9)                        # [c, t]
    wf = np.ascontiguousarray(
        (pw_w.T[None, :, :] * dwt.T[:, :, None]).transpose(1, 0, 2)
    ).astype(np.float32)                            # [c, t, o]
    b1 = (pw_w @ dw_b + pw_b).astype(np.float32).reshape(C, 1)
    ow = np.ascontiguousarray(
        off_w.reshape(18, C, 9).transpose(1, 2, 0)).astype(bf)   # [c, t, 18]
    obr = np.broadcast_to(off_b[None, :], (128, 18)).astype(np.float32)
    obr = np.ascontiguousarray(obr)
    wdm = np.ascontiguousarray(
        de_w.reshape(C, C, 9).transpose(1, 2, 0)).astype(bf)     # [c, k, o]
    dbc = de_b.reshape(C, 1).astype(np.float32)
    gwc = gn_w.reshape(C, 1).astype(np.float32)
    gbc = gn_b.reshape(C, 1).astype(np.float32)
    idn = np.eye(128, dtype=bf)
    onec = np.ones((C, 1), np.float32)
    # iota: w + ZX*(8 + r + ky) + (kx + 8), k = (ky+1)*3 + (kx+1)
    wv = np.arange(96)[:, None, None]
    rv = np.arange(RPC)[None, :, None]
    kyv = (np.arange(9) // 3 - 1)[None, None, :]
    kxv = (np.arange(9) % 3 - 1)[None, None, :]
    iotc = (wv + ZX * (8 + rv + kyv) + kxv + 8).astype(np.float32)

    in_maps = []
    for core in range(NCORES):
        b = core // 2
        r0 = (core % 2) * RPC
        xp = np.zeros((C, XR, XC), np.float32)
        glo, ghi = max(0, r0 - 9), min(H, r0 + 57)
        xp[:, glo - (r0 - 9):ghi - (r0 - 9), 9:105] = x[b, :, glo:ghi, :]
        vrow = np.zeros((C, GR), bf)
        vlo, vhi = max(0, r0 - 8), min(H, r0 + 56)
        vrow[:, vlo - (r0 - 8):vhi - (r0 - 8)] = bf(1.0)
        in_maps.append({
            "x_sh": xp, "vrow": vrow, "wf": wf, "b1c": b1, "ow": ow,
            "obr": obr, "wd": wdm, "dbc": dbc, "gwc": gwc, "gbc": gbc,
            "idn": idn, "onec": onec, "iotc": iotc,
        })
    return in_maps


def get_program():
    if "nc" not in _CACHE:
        _CACHE["nc"] = _build_program()
    return _CACHE["nc"]


def kernel(**inputs):
    nc = get_program()
    in_maps = _prep_inputs(inputs)
    res = run_bass_kernel_spmd(nc, in_maps, core_ids=list(range(NCORES)))
    out = np.empty((B, C, H, W), np.float32)
    for core in range(NCORES):
        b = core // 2
        r0 = (core % 2) * RPC
        out[b, :, r0:r0 + RPC, :] = res.results[core]["out_sh"]
    return out



# revision 9
# speedup vs baseline: 1.9485x; 1.9485x over previous
"""Trainium2 Bass kernel for nn_DASAttentionGate (depthwise-sep conv -> InstanceNorm
-> ReLU -> offset conv -> deformable conv -> GroupNorm -> sigmoid gate).

Sharding: 8 cores = 4 samples x 2 H-halves (48 output rows each). Cross-core
communication: two tiny AllReduces (InstanceNorm + GroupNorm statistics) within
sample pairs.

Deformable conv ("column-pair gather", v2 — no DRAM staging):
  - h_n transposed to pixel-major h_T; a DRAM table z2 of column PAIRS:
    z2[(y,x)] = [h(y,x), h(y+1,x)] (256 bf16 = 512B). A gather of 512 elements
    with elem_step=256 at index (y,x) returns the full bilinear quad
    [h(y,x), h(y+1,x), h(y,x+1), h(y+1,x+1)] from two adjacent entries.
  - Offsets are computed per-row ([w, r, 18]) then PE-transposed into
    pixel-major [128, NT, 18]; bilinear masks, corner weights, and gather
    indices are all computed directly in pixel-major layout — no DRAM
    round-trip.
  - The wrapped-16 gather index tiles are produced with 8 selection matmuls
    (128->16 partition fold) + int16 copies, then partition-replicated.
  - gpsimd.dma_gather fetches one 1KB quad per (tap, pixel); DVE scales the
    4 corners (bf16 2x via duplicated-mask APs); PE sums corners + transposes
    back to channel-major via 4 accumulating identity matmuls, then contracts
    channels with the deform weights, accumulating all 9 taps in PSUM.
"""

import os
import sys

for _p in ("/opt/trn_rl_repo",):
    if os.path.isdir(_p) and _p not in sys.path:
        sys.path.insert(0, _p)

import numpy as np
import ml_dtypes

import concourse.bass as bass
import concourse.bacc as bacc
import concourse.tile as tile
from concourse import mybir
from concourse.ap import AP
from concourse.bass_utils import run_bass_kernel_spmd

F32 = mybir.dt.float32
F32R = mybir.dt.float32r
BF16 = mybir.dt.bfloat16
I16 = mybir.dt.int16
I32 = mybir.dt.int32
AF = mybir.ActivationFunctionType
OP = mybir.AluOpType

# problem geometry (hardcoded per the task contract)
B, C, H, W = 4, 128, 96, 96
NCORES = 8
RPC = 48           # output rows per core
GR, GC = 64, 112   # h grid: rows r0-8..r0+55, cols -8..103
XR, XC = 66, 114   # x grid: rows r0-9..r0+56, cols -9..104
ZY, ZX = 64, 112   # col-pair table grid: y0' in 0..63, x0' in 0..111
NB = ZY * ZX       # 7168 table entries (y=63 zero-filled)
NPIX = RPC * W     # 4608 output pixels per core
NT = NPIX // 128   # 36 pixel tiles
BLKT = 9           # pixel tiles per gather block
NBLK = NT // BLKT  # 4 gather blocks
NIDX = BLKT * 128  # 1152 indices per gather call
NC16 = NIDX // 16  # 72 wrapped-index columns
EPS = 1e-5

_CACHE = {}


def _build_program():
    nc = bacc.Bacc("TRN2", target_bir_lowering=False, debug=False,
                   num_devices=NCORES)

    # ---- I/O ----
    x_d = nc.dram_tensor("x_sh", [C, XR, XC], F32, kind="ExternalInput")
    vm_d = nc.dram_tensor("vrow", [C, GR], BF16, kind="ExternalInput")
    wf_d = nc.dram_tensor("wf", [C, 9, C], F32, kind="ExternalInput")
    b1_d = nc.dram_tensor("b1c", [C, 1], F32, kind="ExternalInput")
    ow_d = nc.dram_tensor("ow", [C, 9, 18], BF16, kind="ExternalInput")
    ob_d = nc.dram_tensor("obr", [128, 18], F32, kind="ExternalInput")
    wd_d = nc.dram_tensor("wd", [C, 9, C], BF16, kind="ExternalInput")
    db_d = nc.dram_tensor("dbc", [C, 1], F32, kind="ExternalInput")
    gw_d = nc.dram_tensor("gwc", [C, 1], F32, kind="ExternalInput")
    gb_d = nc.dram_tensor("gbc", [C, 1], F32, kind="ExternalInput")
    id_d = nc.dram_tensor("idn", [128, 128], BF16, kind="ExternalInput")
    if_d = nc.dram_tensor("idnf", [128, 128], F32, kind="ExternalInput")
    on_d = nc.dram_tensor("onec", [C, 1], F32, kind="ExternalInput")
    io_d = nc.dram_tensor("iotc", [128, NT, 9], F32, kind="ExternalInput")
    out_d = nc.dram_tensor("out_sh", [C, RPC, W], F32, kind="ExternalOutput")

    groups = [[0, 1], [2, 3], [4, 5], [6, 7]]

    # (tile, row, w0, p0) 32-wide pieces for the pixel-major transpose:
    # pixel = r*96 + w lands in tile g at local partition p = pixel - g*128.
    # All offsets are multiples of 32 (gcd(96,128)=32), matching the PE's
    # 32x32 tile_position granularity.
    pieces = []
    for g in range(NT):
        r_lo = (g * 128) // 96
        r_hi = (g * 128 + 127) // 96
        for r in range(r_lo, r_hi + 1):
            w0 = max(0, g * 128 - r * 96)
            w1 = min(96, (g + 1) * 128 - r * 96)
            for wc in range(w0, w1, 32):
                pieces.append((g, r, wc, r * 96 + wc - g * 128))

    with tile.TileContext(nc) as tc:
        with (
            tc.tile_pool(name="const", bufs=1) as constp,
            tc.tile_pool(name="xbuf", bufs=1) as xpool,
            tc.tile_pool(name="hbuf", bufs=1) as hpool,
            tc.tile_pool(name="mwork", bufs=1) as mpool,
            tc.tile_pool(name="gbuf", bufs=1) as gpool,
            tc.tile_pool(name="sbig", bufs=1) as spool,
            tc.tile_pool(name="ps", bufs=6, space="PSUM") as psp,
            tc.tile_pool(name="dram", bufs=1, space="DRAM") as dramp,
        ):
            # ---- load constants ----
            wf = constp.tile([C, 9, C], F32R, tag="wf")
            nc.sync.dma_start(wf[:], wf_d[:].bitcast(F32R))
            b1 = constp.tile([C, 1], F32, tag="b1")
            nc.sync.dma_start(b1[:], b1_d[:])
            ow = constp.tile([C, 9, 18], BF16, tag="ow")
            nc.sync.dma_start(ow[:], ow_d[:])
            ob = constp.tile([128, 18], F32, tag="ob")
            nc.sync.dma_start(ob[:], ob_d[:])
            wd = constp.tile([C, 9, C], BF16, tag="wd")
            nc.sync.dma_start(wd[:], wd_d[:])
            db = constp.tile([C, 1], F32, tag="db")
            nc.sync.dma_start(db[:], db_d[:])
            gw = constp.tile([C, 1], F32, tag="gw")
            nc.sync.dma_start(gw[:], gw_d[:])
            gb = constp.tile([C, 1], F32, tag="gb")
            nc.sync.dma_start(gb[:], gb_d[:])
            idn = constp.tile([128, 128], BF16, tag="idn")
            nc.sync.dma_start(idn[:], id_d[:])
            idnf = constp.tile([128, 128], F32, tag="idnf")
            nc.sync.dma_start(idnf[:], if_d[:])
            onec = constp.tile([C, 1], F32, tag="onec")
            nc.sync.dma_start(onec[:], on_d[:])
            vm = constp.tile([C, GR], BF16, tag="vm")
            nc.sync.dma_start(vm[:], vm_d[:])
            iot = constp.tile([128, NT, 9], F32, tag="iot")
            nc.sync.dma_start(iot[:], io_d[:])
            zrow = constp.tile([112, 256], BF16, tag="zrow")
            nc.gpsimd.memset(zrow[:], 0.0)

            # ---- conv1 (fused depthwise+pointwise, f32r) ----
            xs = xpool.tile([C, XR, XC], F32R, tag="xs")
            nc.sync.dma_start(xs[:, 0:33, :], x_d[:, 0:33, :].bitcast(F32R))
            nc.sync.dma_start(xs[:, 33:66, :], x_d[:, 33:66, :].bitcast(F32R))

            hraw = hpool.tile([C, GR, GC], F32, tag="hraw")
            CH = 4  # grid rows per psum chunk
            for ch in range(GR // CH):
                gr0 = ch * CH
                pt = psp.tile([128, CH * GC], F32, tag="ps")
                for t in range(9):
                    ty, tx = t // 3, t % 3
                    rhs = xs[:, gr0 + ty:gr0 + ty + CH, tx:tx + GC]
                    nc.tensor.matmul(
                        pt[:], wf[:, t, :], rhs,
                        start=(t == 0), stop=(t == 8))
                nc.scalar.activation(
                    hraw[:, gr0:gr0 + CH, :].rearrange("p a b -> p (a b)"),
                    pt[:], AF.Identity, bias=b1[:])

            # ---- InstanceNorm stats over own 48 valid rows ----
            valid = hraw[:, 8:56, 8:104]
            st = mpool.tile([C, 2], F32, tag="st")
            nc.vector.tensor_reduce(st[:, 0:1], valid, mybir.AxisListType.XY,
                                    OP.add)
            sq = spool.tile([C, NPIX], F32, tag="big")
            nc.scalar.activation(sq[:].rearrange("p (a b) -> p a b", a=RPC),
                                 valid, AF.Square, accum_out=st[:, 1:2])

            cc_in = dramp.tile([C, 2], F32, tag="cci")
            cc_out = dramp.tile([C, 2], F32, tag="cco")
            nc.sync.dma_start(cc_in[:], st[:])
            nc.gpsimd.collective_compute(
                "AllReduce", OP.add, replica_groups=groups,
                ins=[cc_in[:].opt()], outs=[cc_out[:].opt()])
            stg = mpool.tile([C, 2], F32, tag="stg")
            nc.sync.dma_start(stg[:], cc_out[:])

            # mean/rstd per channel
            mom = mpool.tile([C, 2], F32, tag="mom")
            nc.vector.tensor_scalar(mom[:], stg[:], 1.0 / (H * W), None,
                                    OP.mult)
            var = mpool.tile([C, 1], F32, tag="var")
            nc.vector.tensor_tensor(var[:], mom[:, 0:1], mom[:, 0:1], OP.mult)
            nc.vector.tensor_tensor(var[:], mom[:, 1:2], var[:], OP.subtract)
            nc.vector.tensor_scalar(var[:], var[:], EPS, None, OP.add)
            rstd = mpool.tile([C, 1], F32, tag="rstd")
            nc.scalar.activation(rstd[:], var[:], AF.Sqrt)
            nc.vector.reciprocal(rstd[:], rstd[:])
            nbias = mpool.tile([C, 1], F32, tag="nbias")
            nc.vector.tensor_tensor(nbias[:], mom[:, 0:1], rstd[:], OP.mult)
            nc.vector.tensor_scalar(nbias[:], nbias[:], -1.0, None, OP.mult)

            # ---- h_n (bf16, masked) + f32 shortcut ----
            hn = hpool.tile([C, GR, GC], BF16, tag="hn")
            nc.scalar.activation(hn[:], hraw[:], AF.Relu, bias=nbias[:],
                                 scale=rstd[:])
            vmb = vm[:].unsqueeze(2).broadcast_to((C, GR, GC))
            nc.vector.tensor_tensor(hn[:], hn[:], vmb, OP.mult)
            nc.gpsimd.memset(hn[:, :, 0:8], 0.0)
            nc.gpsimd.memset(hn[:, :, 104:112], 0.0)
            short = spool.tile([C, NPIX], F32, tag="short")
            nc.scalar.activation(
                short[:].rearrange("p (a b) -> p a b", a=RPC),
                hraw[:, 8:56, 8:104], AF.Relu, bias=nbias[:], scale=rstd[:])

            # ---- h_T (pixel-major transpose of h_n rows) ----
            hT = hpool.tile([112, GR, 128], BF16, tag="hraw")
            for gr in range(GR):
                pt = psp.tile([112, 128], F32, tag="ps")
                nc.tensor.matmul(pt[:], hn[:, gr, :], idn[:],
                                 start=True, stop=True)
                nc.scalar.activation(hT[:, gr, :], pt[:], AF.Copy)

            # ---- z2 column-pair table in DRAM ----
            # z2[y*112 + x] = [h(y, x) 128ch, h(y+1, x) 128ch]; row y=63 zero.
            # one pad entry so the overlapping 512-elem gather AP stays in
            # bounds.
            z2 = dramp.tile([NB + 1, 256], BF16, tag="z2")
            z2v = z2[0:NB].rearrange("(a p x) c -> p x a c", p=2, x=ZX)
            hTe = hT[:].rearrange("x (a b) c -> x a (b c)", b=2)
            nc.sync.dma_start(z2v[0], hTe)                       # y even
            hTo = hT[:, 1:63, :].rearrange("x (a b) c -> x a (b c)", b=2)
            nc.sync.dma_start(z2v[1][:, 0:31, :], hTo)           # y odd
            # y = 63: [h(63, x), zeros] (row 64 is outside the grid)
            nc.sync.dma_start(z2v[1][:, 31, 0:128], hT[:, 63, :])
            nc.sync.dma_start(z2v[1][:, 31, 128:256], zrow[:, 0:128])

            # ---- offset conv (bf16, [w, r, 18], bias added post-transpose) --
            offT = mpool.tile([96, RPC, 18], F32, tag="offT")
            for r in range(RPC):
                gr = r + 8
                po = psp.tile([96, 18], F32, tag="ps")
                for t in range(9):
                    ty, tx = t // 3, t % 3
                    lhsT = hn[:, gr + ty - 1, 7 + tx:7 + tx + 96]
                    nc.tensor.matmul(po[:], lhsT, ow[:, t, :],
                                     start=(t == 0), stop=(t == 8))
                nc.scalar.activation(offT[:, r, :], po[:], AF.Copy)

            # ---- pixel-major offsets offP[128, NT, 18] via PE transpose ----
            offP = mpool.tile([128, NT, 18], F32, tag="offP")
            g_cur = -1
            pog = None
            for (g, r, w0, p0) in pieces:
                if g != g_cur:
                    if g_cur >= 0:
                        nc.scalar.activation(offP[:, g_cur, :], pog[:],
                                             AF.Copy)
                    pog = psp.tile([128, 18], F32, tag="ps")
                    g_cur = g
                nc.tensor.matmul(pog[p0:p0 + 32, :],
                                 idnf[w0:w0 + 32, w0:w0 + 32],
                                 offT[w0:w0 + 32, r, :],
                                 start=True, stop=True,
                                 tile_position=(w0, p0))
            nc.scalar.activation(offP[:, g_cur, :], pog[:], AF.Copy)
            obv = ob[:].unsqueeze(1).broadcast_to((128, NT, 18))
            nc.vector.tensor_tensor(offP[:], offP[:], obv, OP.add)

            # ---- bilinear masks + gather indices (pixel-major) ----
            def mk(tag):
                return mpool.tile([128, NT, 9], F32, tag=tag, name=tag)

            offv = offP[:].rearrange("p g (k two) -> p g k two", two=2)
            oy, ox = offv[:, :, :, 0], offv[:, :, :, 1]
            it32 = mpool.tile([128, NT, 9], I32, tag="it32")
            kf = mk("kf")
            gt = mk("gt")
            fy = mk("fy")
            ly = mk("ly")
            fx = mk("fx")
            lx = mk("lx")
            for (o_, f_, l_) in ((oy, fy, ly), (ox, fx, lx)):
                nc.vector.tensor_copy(it32[:], o_)
                nc.vector.tensor_copy(kf[:], it32[:])
                nc.vector.tensor_tensor(gt[:], kf[:], o_, OP.is_gt)
                nc.vector.tensor_tensor(f_[:], kf[:], gt[:], OP.subtract)
                nc.vector.tensor_tensor(l_[:], o_, f_[:], OP.subtract)
            uy = mk("uy")
            ux = mk("ux")
            nc.vector.tensor_scalar(uy[:], ly[:], -1.0, 1.0, OP.mult, OP.add)
            nc.vector.tensor_scalar(ux[:], lx[:], -1.0, 1.0, OP.mult, OP.add)
            # corner weights (col-pair order: y0x0, y1x0, y0x1, y1x1),
            # duplicated pairs for bf16 2x vector mode
            a_w = gpool.tile([128, NT, 9, 4, 2], BF16, tag="a_w")

            def dup2(ap):
                return ap.unsqueeze(3).broadcast_to((128, NT, 9, 2))

            for j, (fa, fb) in enumerate(((uy, ux), (ly, ux),
                                          (uy, lx), (ly, lx))):
                nc.vector.tensor_tensor(a_w[:, :, :, j, :],
                                        dup2(fa[:]), dup2(fb[:]), OP.mult)

            # idx = iota + 112*fy + fx, clamped to [0, NB-2] (keeps both
            # gathered entries in the written region; clamped cases only
            # ever touch zeroed h columns/rows)
            idxf = mk("idxf")
            nc.vector.tensor_scalar(idxf[:], fy[:], float(ZX), None, OP.mult)
            nc.vector.tensor_tensor(idxf[:], idxf[:], fx[:], OP.add)
            nc.vector.tensor_tensor(idxf[:], idxf[:], iot[:], OP.add)
            nc.vector.tensor_scalar(idxf[:], idxf[:], 0.0, float(NB - 2),
                                    OP.max, OP.min)

            # ---- wrapped-16 idx tiles via 8 selection matmuls ----
            # idx_w[p, kk*NBLK+blk, tb*8+cc] = idx[pixel (blk*9+tb)*128 +
            # cc*16+p]
            idx_w = gpool.tile([128, 9 * NBLK, NC16], I16, tag="idx_w")
            idx_wv = idx_w[0:16].rearrange(
                "p (kk blk) (tb cc) -> p kk blk tb cc", kk=9, cc=8)
            rhsx = idxf[:].rearrange("p a b -> p (a b)")
            for cc in range(8):
                pcc = psp.tile([16, NT * 9], F32, tag="ps")
                nc.tensor.matmul(pcc[:], idnf[:, cc * 16:cc * 16 + 16],
                                 rhsx, start=True, stop=True)
                src = pcc[:].rearrange("p (blk tb kk) -> p kk blk tb",
                                       blk=NBLK, kk=9)
                nc.vector.tensor_copy(idx_wv[:, :, :, :, cc], src)
            nc.sync.dma_start(idx_w[16:32, :, :], idx_w[0:16, :, :])
            nc.sync.dma_start(idx_w[32:64, :, :], idx_w[0:32, :, :])
            nc.sync.dma_start(idx_w[64:128, :, :], idx_w[0:64, :, :])

            # overlapping gather source AP: entry stride 256 elems, but each
            # gather reads 512 elems (two adjacent column-pair entries)
            z2ap = z2[:]
            gsrc = AP(z2ap.tensor, z2ap.offset, [[256, NB], [1, 512]])

            # ---- gather + scale + corner-sum/transpose + einsum ----
            d_sb = spool.tile([C, NT, 128], F32, tag="dsb")
            gst = mpool.tile([C, NBLK, 2], F32, tag="gst")
            for blk in range(NBLK):
                sampT = xpool.tile([128, 9, BLKT, 128], BF16, tag="xs")
                for kk in range(9):
                    g_t = gpool.tile([128, BLKT, 4, 128], BF16, tag="g_t",
                                     bufs=2)
                    nc.gpsimd.dma_gather(
                        g_t[:].rearrange("p a b c -> p a (b c)"),
                        gsrc, idx_w[:, kk * NBLK + blk, :],
                        NIDX, NIDX, 512, elem_step=256, queue_num=0,
                        single_packet=False)
                    # scale corners by bilinear weights (bf16 2x, dup pairs)
                    gv = g_t[:].rearrange("p a b (c two) -> p a b c two",
                                          two=2)
                    for j in range(4):
                        av = a_w[:, blk * BLKT:(blk + 1) * BLKT, kk, j, :]
                        av = av.unsqueeze(2).broadcast_to((128, BLKT, 64, 2))
                        nc.vector.tensor_tensor(gv[:, :, j], gv[:, :, j], av,
                                                OP.mult)
                    # sum 4 corners + transpose to channel-major via PE
                    for t in range(BLKT):
                        pt = psp.tile([128, 128], F32, tag="ps")
                        for j in range(4):
                            nc.tensor.matmul(pt[:], g_t[:, t, j, :], idn[:],
                                             start=(j == 0), stop=(j == 3))
                        nc.scalar.activation(sampT[:, kk, t, :], pt[:],
                                             AF.Copy)
                # einsum: accumulate 9 taps
                for t in range(BLKT):
                    pd = psp.tile([128, 128], F32, tag="ps")
                    for kk in range(9):
                        nc.tensor.matmul(pd[:], wd[:, kk, :],
                                         sampT[:, kk, t, :],
                                         start=(kk == 0), stop=(kk == 8))
                    nc.scalar.activation(d_sb[:, blk * BLKT + t, :], pd[:],
                                         AF.Identity, bias=db[:])
                # per-block GroupNorm partial stats (overlaps the next block)
                dblk = d_sb[:, blk * BLKT:(blk + 1) * BLKT, :]
                nc.vector.tensor_reduce(gst[:, blk, 0:1], dblk,
                                        mybir.AxisListType.XY, OP.add)
                nc.scalar.activation(
                    sq[:, blk * BLKT * 128:(blk + 1) * BLKT * 128].rearrange(
                        "p (a b) -> p a b", a=BLKT),
                    dblk, AF.Square,
                    accum_out=gst[:, blk, 1:2])

            # ---- GroupNorm stats (whole sample) ----
            gss = mpool.tile([C, 2], F32, tag="gss")
            nc.vector.tensor_tensor(gss[:], gst[:, 0, :], gst[:, 1, :],
                                    OP.add)
            nc.vector.tensor_tensor(gss[:], gss[:], gst[:, 2, :], OP.add)
            nc.vector.tensor_tensor(gss[:], gss[:], gst[:, 3, :], OP.add)
            pg = psp.tile([1, 2], F32, tag="ps")
            nc.tensor.matmul(pg[:], onec[:], gss[:], start=True, stop=True)
            gred = mpool.tile([1, 2], F32, tag="gred")
            nc.scalar.activation(gred[:], pg[:], AF.Copy)
            ccg_in = dramp.tile([1, 2], F32, tag="ccgi")
            ccg_out = dramp.tile([1, 2], F32, tag="ccgo")
            nc.sync.dma_start(ccg_in[:], gred[:])
            nc.gpsimd.collective_compute(
                "AllReduce", OP.add, replica_groups=groups,
                ins=[ccg_in[:].opt()], outs=[ccg_out[:].opt()])
            gsc = mpool.tile([1, 2], F32, tag="gsc")
            nc.sync.dma_start(gsc[:], ccg_out[:])
            gall = mpool.tile([128, 2], F32, tag="gall")
            nc.gpsimd.partition_broadcast(gall[:], gsc[:], 128)

            gmom = mpool.tile([C, 2], F32, tag="gmom")
            nc.vector.tensor_scalar(gmom[:], gall[:], 1.0 / (C * H * W), None,
                                    OP.mult)
            gvar = mpool.tile([C, 1], F32, tag="gvar")
            nc.vector.tensor_tensor(gvar[:], gmom[:, 0:1], gmom[:, 0:1],
                                    OP.mult)
            nc.vector.tensor_tensor(gvar[:], gmom[:, 1:2], gvar[:],
                                    OP.subtract)
            nc.vector.tensor_scalar(gvar[:], gvar[:], EPS, None, OP.add)
            grstd = mpool.tile([C, 1], F32, tag="grstd")
            nc.scalar.activation(grstd[:], gvar[:], AF.Sqrt)
            nc.vector.reciprocal(grstd[:], grstd[:])
            # scale2 = gn_w * rstd ; bias2 = gn_b - mean * scale2
            sc2 = mpool.tile([C, 1], F32, tag="sc2")
            nc.vector.tensor_tensor(sc2[:], gw[:], grstd[:], OP.mult)
            bi2 = mpool.tile([C, 1], F32, tag="bi2")
            nc.vector.tensor_tensor(bi2[:], gmom[:, 0:1], sc2[:], OP.mult)
            nc.vector.tensor_tensor(bi2[:], gb[:], bi2[:], OP.subtract)

            # ---- gate + residual ----
            gg = spool.tile([C, NPIX], F32, tag="big")  # reuse sq slot
            nc.scalar.activation(gg[:].rearrange("p (a b) -> p a b", a=NT),
                                 d_sb[:], AF.Sigmoid, bias=bi2[:],
                                 scale=sc2[:])
            nc.vector.tensor_scalar(gg[:], gg[:], 1.0, None, OP.add)
            nc.vector.tensor_tensor(gg[:], gg[:], short[:], OP.mult)
            nc.sync.dma_start(
                out_d[:], gg[:].rearrange("p (r w) -> p r w", w=W))

    nc.compile()
    return nc


def _prep_inputs(inputs):
    x = np.asarray(inputs["x"], np.float32)
    dw_w = np.asarray(inputs["dw_w"], np.float32)
    dw_b = np.asarray(inputs["dw_b"], np.float32)
    pw_w = np.asarray(inputs["pw_w"], np.float32)
    pw_b = np.asarray(inputs["pw_b"], np.float32)
    off_w = np.asarray(inputs["off_w"], np.float32)
    off_b = np.asarray(inputs["off_b"], np.float32)
    de_w = np.asarray(inputs["de_w"], np.float32)
    de_b = np.asarray(inputs["de_b"], np.float32)
    gn_w = np.asarray(inputs["gn_w"], np.float32)
    gn_b = np.asarray(inputs["gn_b"], np.float32)

    bf = ml_dtypes.bfloat16
    # fused conv1 weights: wf[t][c, o] = pw_w[o, c] * dw_w[c, 0, ty, tx]
    dwt = dw_w.reshape(C, 9)                        # [c, t]
    wf = np.ascontiguousarray(
        (pw_w.T[None, :, :] * dwt.T[:, :, None]).transpose(1, 0, 2)
    ).astype(np.float32)                            # [c, t, o]
    b1 = (pw_w @ dw_b + pw_b).astype(np.float32).reshape(C, 1)
    ow = np.ascontiguousarray(
        off_w.reshape(18, C, 9).transpose(1, 2, 0)).astype(bf)   # [c, t, 18]
    obr = np.broadcast_to(off_b[None, :], (128, 18)).astype(np.float32)
    obr = np.ascontiguousarray(obr)
    wdm = np.ascontiguousarray(
        de_w.reshape(C, C, 9).transpose(1, 2, 0)).astype(bf)     # [c, k, o]
    dbc = de_b.reshape(C, 1).astype(np.float32)
    gwc = gn_w.reshape(C, 1).astype(np.float32)
    gbc = gn_b.reshape(C, 1).astype(np.float32)
    idn = np.eye(128, dtype=bf)
    idnf = np.eye(128, dtype=np.float32)
    onec = np.ones((C, 1), np.float32)
    # pixel-major iota: pixel = g*128 + p, (r, w) = divmod(pixel, 96);
    # base entry = (8 + r + ky)*112 + (8 + w + kx)
    pixv = np.arange(NPIX)
    rv = (pixv // 96).reshape(NT, 128)
    wv = (pixv % 96).reshape(NT, 128)
    kyv = (np.arange(9) // 3 - 1)
    kxv = (np.arange(9) % 3 - 1)
    iotc = (ZX * (8 + rv[:, :, None] + kyv[None, None, :])
            + 8 + wv[:, :, None] + kxv[None, None, :])
    iotc = np.ascontiguousarray(
        iotc.transpose(1, 0, 2)).astype(np.float32)  # [128, NT, 9]

    in_maps = []
    for core in range(NCORES):
        b = core // 2
        r0 = (core % 2) * RPC
        xp = np.zeros((C, XR, XC), np.float32)
        glo, ghi = max(0, r0 - 9), min(H, r0 + 57)
        xp[:, glo - (r0 - 9):ghi - (r0 - 9), 9:105] = x[b, :, glo:ghi, :]
        vrow = np.zeros((C, GR), bf)
        vlo, vhi = max(0, r0 - 8), min(H, r0 + 56)
        vrow[:, vlo - (r0 - 8):vhi - (r0 - 8)] = bf(1.0)
        in_maps.append({
            "x_sh": xp, "vrow": vrow, "wf": wf, "b1c": b1, "ow": ow,
            "obr": obr, "wd": wdm, "dbc": dbc, "gwc": gwc, "gbc": gbc,
            "idn": idn, "idnf": idnf, "onec": onec, "iotc": iotc,
        })
    return in_maps


def get_program():
    if "nc" not in _CACHE:
        _CACHE["nc"] = _build_program()
    return _CACHE["nc"]


def kernel(**inputs):
    nc = get_program()
    in_maps = _prep_inputs(inputs)
    res = run_bass_kernel_spmd(nc, in_maps, core_ids=list(range(NCORES)))
    out = np.empty((B, C, H, W), np.float32)
    for core in range(NCORES):
        b = core // 2
        r0 = (core % 2) * RPC
        out[b, :, r0:r0 + RPC, :] = res.results[core]["out_sh"]
    return out


# revision 14
# speedup vs baseline: 1.9856x; 1.0190x over previous
"""Trainium2 Bass kernel for nn_DASAttentionGate (depthwise-sep conv -> InstanceNorm
-> ReLU -> offset conv -> deformable conv -> GroupNorm -> sigmoid gate).

Sharding: 8 cores = 4 samples x 2 H-halves (48 output rows each). Cross-core
communication: two tiny AllReduces (InstanceNorm + GroupNorm statistics) within
sample pairs.

Deformable conv ("column-pair gather", v2 — no DRAM staging):
  - h_n transposed to pixel-major h_T; a DRAM table z2 of column PAIRS:
    z2[(y,x)] = [h(y,x), h(y+1,x)] (256 bf16 = 512B). A gather of 512 elements
    with elem_step=256 at index (y,x) returns the full bilinear quad
    [h(y,x), h(y+1,x), h(y,x+1), h(y+1,x+1)] from two adjacent entries.
  - Offsets are computed per-row ([w, r, 18]) then PE-transposed into
    pixel-major [128, NT, 18]; bilinear masks, corner weights, and gather
    indices are all computed directly in pixel-major layout — no DRAM
    round-trip.
  - The wrapped-16 gather index tiles are produced with 8 selection matmuls
    (128->16 partition fold) + int16 copies, then partition-replicated.
  - gpsimd.dma_gather fetches one 1KB quad per (tap, pixel); DVE scales the
    4 corners (bf16 2x via duplicated-mask APs); PE sums corners + transposes
    back to channel-major via 4 accumulating identity matmuls, then contracts
    channels with the deform weights, accumulating all 9 taps in PSUM.
"""

import os
import sys

for _p in ("/opt/trn_rl_repo",):
    if os.path.isdir(_p) and _p not in sys.path:
        sys.path.insert(0, _p)

import numpy as np
import ml_dtypes

import concourse.bass as bass
import concourse.bacc as bacc
import concourse.tile as tile
from concourse import mybir
from concourse.ap import AP
from concourse.bass_utils import run_bass_kernel_spmd

F32 = mybir.dt.float32
F32R = mybir.dt.float32r
BF16 = mybir.dt.bfloat16
I16 = mybir.dt.int16
I32 = mybir.dt.int32
AF = mybir.ActivationFunctionType
OP = mybir.AluOpType

# problem geometry (hardcoded per the task contract)
B, C, H, W = 4, 128, 96, 96
NCORES = 8
RPC = 48           # output rows per core
GR, GC = 64, 112   # h grid: rows r0-8..r0+55, cols -8..103
XR, XC = 66, 114   # x grid: rows r0-9..r0+56, cols -9..104
ZY, ZX = 64, 112   # col-pair table grid: y0' in 0..63, x0' in 0..111
NB = ZY * ZX       # 7168 table entries (y=63 zero-filled)
NPIX = RPC * W     # 4608 output pixels per core
NT = NPIX // 128   # 36 pixel tiles
BLKT = 9           # pixel tiles per gather block
NBLK = NT // BLKT  # 4 gather blocks
NIDX = BLKT * 128  # 1152 indices per gather call
NC16 = NIDX // 16  # 72 wrapped-index columns
EPS = 1e-5

_CACHE = {}


def _build_program():
    nc = bacc.Bacc("TRN2", target_bir_lowering=False, debug=False,
                   num_devices=NCORES)

    # ---- I/O ----
    x_d = nc.dram_tensor("x_sh", [C, XR, XC], F32, kind="ExternalInput")
    vm_d = nc.dram_tensor("vrow", [C, GR], BF16, kind="ExternalInput")
    wf_d = nc.dram_tensor("wf", [C, 9, C], F32, kind="ExternalInput")
    b1_d = nc.dram_tensor("b1c", [C, 1], F32, kind="ExternalInput")
    ow_d = nc.dram_tensor("ow", [C, 9, 18], BF16, kind="ExternalInput")
    ob_d = nc.dram_tensor("obr", [128, 18], F32, kind="ExternalInput")
    wd_d = nc.dram_tensor("wd", [C, 9, C], BF16, kind="ExternalInput")
    db_d = nc.dram_tensor("dbc", [C, 1], F32, kind="ExternalInput")
    gw_d = nc.dram_tensor("gwc", [C, 1], F32, kind="ExternalInput")
    gb_d = nc.dram_tensor("gbc", [C, 1], F32, kind="ExternalInput")
    id_d = nc.dram_tensor("idn", [128, 128], BF16, kind="ExternalInput")
    if_d = nc.dram_tensor("idnf", [128, 128], F32, kind="ExternalInput")
    on_d = nc.dram_tensor("onec", [C, 1], F32, kind="ExternalInput")
    os_d = nc.dram_tensor("onesf", [C, 128], F32, kind="ExternalInput")
    io_d = nc.dram_tensor("iotc", [128, NT, 9], F32, kind="ExternalInput")
    out_d = nc.dram_tensor("out_sh", [C, RPC, W], F32, kind="ExternalOutput")

    groups = [[0, 1], [2, 3], [4, 5], [6, 7]]

    # (tile, row, w0, p0) 32-wide pieces for the pixel-major transpose:
    # pixel = r*96 + w lands in tile g at local partition p = pixel - g*128.
    # All offsets are multiples of 32 (gcd(96,128)=32), matching the PE's
    # 32x32 tile_position granularity.
    pieces = []
    for g in range(NT):
        r_lo = (g * 128) // 96
        r_hi = (g * 128 + 127) // 96
        for r in range(r_lo, r_hi + 1):
            w0 = max(0, g * 128 - r * 96)
            w1 = min(96, (g + 1) * 128 - r * 96)
            for wc in range(w0, w1, 32):
                pieces.append((g, r, wc, r * 96 + wc - g * 128))

    with tile.TileContext(nc) as tc:
        with (
            tc.tile_pool(name="const", bufs=1) as constp,
            tc.tile_pool(name="xbuf", bufs=1) as xpool,
            tc.tile_pool(name="hbuf", bufs=1) as hpool,
            tc.tile_pool(name="mwork", bufs=1) as mpool,
            tc.tile_pool(name="gbuf", bufs=1) as gpool,
            tc.tile_pool(name="sbig", bufs=1) as spool,
            tc.tile_pool(name="ps", bufs=6, space="PSUM") as psp,
            tc.tile_pool(name="dram", bufs=1, space="DRAM") as dramp,
        ):
            # ---- load constants ----
            wf = constp.tile([C, 9, C], F32R, tag="wf")
            nc.sync.dma_start(wf[:], wf_d[:].bitcast(F32R))
            b1 = constp.tile([C, 1], F32, tag="b1")
            nc.sync.dma_start(b1[:], b1_d[:])
            ow = constp.tile([C, 9, 18], BF16, tag="ow")
            nc.sync.dma_start(ow[:], ow_d[:])
            ob = constp.tile([128, 18], F32, tag="ob")
            nc.sync.dma_start(ob[:], ob_d[:])
            wd = constp.tile([C, 9, C], BF16, tag="wd")
            nc.sync.dma_start(wd[:], wd_d[:])
            db = constp.tile([C, 1], F32, tag="db")
            nc.sync.dma_start(db[:], db_d[:])
            gw = constp.tile([C, 1], F32, tag="gw")
            nc.sync.dma_start(gw[:], gw_d[:])
            gb = constp.tile([C, 1], F32, tag="gb")
            nc.sync.dma_start(gb[:], gb_d[:])
            idn = constp.tile([128, 128], BF16, tag="idn")
            nc.sync.dma_start(idn[:], id_d[:])
            idnf = constp.tile([128, 128], F32, tag="idnf")
            nc.sync.dma_start(idnf[:], if_d[:])
            onec = constp.tile([C, 1], F32, tag="onec")
            nc.sync.dma_start(onec[:], on_d[:])
            onesf = constp.tile([C, 128], F32, tag="onesf")
            nc.sync.dma_start(onesf[:], os_d[:])
            vm = constp.tile([C, GR], BF16, tag="vm")
            nc.sync.dma_start(vm[:], vm_d[:])
            iot = constp.tile([128, NT, 9], F32, tag="iot")
            nc.sync.dma_start(iot[:], io_d[:])
            zrow = constp.tile([112, 256], BF16, tag="zrow")
            nc.gpsimd.memset(zrow[:], 0.0)

            # ---- conv1 (fused depthwise+pointwise, f32r) ----
            # middle rows first so InstanceNorm stats + AllReduce can start
            # while the halo rows are still being computed.
            xs = xpool.tile([C, XR, XC], F32R, tag="xs")
            nc.sync.dma_start(xs[:, 8:33, :], x_d[:, 8:33, :].bitcast(F32R))
            nc.sync.dma_start(xs[:, 33:59, :], x_d[:, 33:59, :].bitcast(F32R))
            nc.sync.dma_start(xs[:, 0:8, :], x_d[:, 0:8, :].bitcast(F32R))
            nc.sync.dma_start(xs[:, 59:66, :], x_d[:, 59:66, :].bitcast(F32R))

            hraw = hpool.tile([C, GR, GC], F32, tag="hraw")
            CH = 4  # grid rows per psum chunk

            def conv1_chunk(ch):
                gr0 = ch * CH
                pt = psp.tile([128, CH * GC], F32, tag="ps")
                for t in range(9):
                    ty, tx = t // 3, t % 3
                    rhs = xs[:, gr0 + ty:gr0 + ty + CH, tx:tx + GC]
                    nc.tensor.matmul(
                        pt[:], wf[:, t, :], rhs,
                        start=(t == 0), stop=(t == 8))
                nc.scalar.activation(
                    hraw[:, gr0:gr0 + CH, :].rearrange("p a b -> p (a b)"),
                    pt[:], AF.Identity, bias=b1[:])

            for ch in range(2, 14):
                conv1_chunk(ch)

            # ---- InstanceNorm stats over own 48 valid rows ----
            valid = hraw[:, 8:56, 8:104]
            st = mpool.tile([C, 2], F32, tag="st")
            nc.vector.tensor_reduce(st[:, 0:1], valid, mybir.AxisListType.XY,
                                    OP.add)
            sq = spool.tile([C, NPIX], F32, tag="big")
            nc.scalar.activation(sq[:].rearrange("p (a b) -> p a b", a=RPC),
                                 valid, AF.Square, accum_out=st[:, 1:2])

            cc_in = dramp.tile([C, 2], F32, tag="cci")
            cc_out = dramp.tile([C, 2], F32, tag="cco")
            nc.sync.dma_start(cc_in[:], st[:])
            nc.gpsimd.collective_compute(
                "AllReduce", OP.add, replica_groups=groups,
                ins=[cc_in[:].opt()], outs=[cc_out[:].opt()])

            # halo rows overlap the AllReduce
            for ch in (0, 1, 14, 15):
                conv1_chunk(ch)

            stg = mpool.tile([C, 2], F32, tag="stg")
            nc.sync.dma_start(stg[:], cc_out[:])

            # mean/rstd per channel
            mom = mpool.tile([C, 2], F32, tag="mom")
            nc.vector.tensor_scalar(mom[:], stg[:], 1.0 / (H * W), None,
                                    OP.mult)
            var = mpool.tile([C, 1], F32, tag="var")
            nc.vector.tensor_tensor(var[:], mom[:, 0:1], mom[:, 0:1], OP.mult)
            nc.vector.tensor_tensor(var[:], mom[:, 1:2], var[:], OP.subtract)
            nc.vector.tensor_scalar(var[:], var[:], EPS, None, OP.add)
            rstd = mpool.tile([C, 1], F32, tag="rstd")
            nc.scalar.activation(rstd[:], var[:], AF.Sqrt)
            nc.vector.reciprocal(rstd[:], rstd[:])
            nbias = mpool.tile([C, 1], F32, tag="nbias")
            nc.vector.tensor_tensor(nbias[:], mom[:, 0:1], rstd[:], OP.mult)
            nc.vector.tensor_scalar(nbias[:], nbias[:], -1.0, None, OP.mult)

            # ---- h_n (bf16, masked) + f32 shortcut ----
            hn = hpool.tile([C, GR, GC], BF16, tag="hn")
            nc.scalar.activation(hn[:], hraw[:], AF.Relu, bias=nbias[:],
                                 scale=rstd[:])
            vmb = vm[:].unsqueeze(2).broadcast_to((C, GR, GC))
            nc.vector.tensor_tensor(hn[:], hn[:], vmb, OP.mult)
            nc.gpsimd.memset(hn[:, :, 0:8], 0.0)
            nc.gpsimd.memset(hn[:, :, 104:112], 0.0)
            short = spool.tile([C, NPIX], F32, tag="short")
            nc.scalar.activation(
                short[:].rearrange("p (a b) -> p a b", a=RPC),
                hraw[:, 8:56, 8:104], AF.Relu, bias=nbias[:], scale=rstd[:])

            # ---- h_T (pixel-major transpose of h_n rows) ----
            hT = hpool.tile([112, GR, 128], BF16, tag="hraw")
            for gr in range(GR):
                pt = psp.tile([112, 128], F32, tag="ps")
                nc.tensor.matmul(pt[:], hn[:, gr, :], idn[:],
                                 start=True, stop=True)
                nc.scalar.activation(hT[:, gr, :], pt[:], AF.Copy)

            # ---- z2 column-pair table in DRAM ----
            # z2[y*112 + x] = [h(y, x) 128ch, h(y+1, x) 128ch]; row y=63 zero.
            # one pad entry so the overlapping 512-elem gather AP stays in
            # bounds.
            z2 = dramp.tile([NB + 1, 256], BF16, tag="z2")
            z2v = z2[0:NB].rearrange("(a p x) c -> p x a c", p=2, x=ZX)
            hTe = hT[:].rearrange("x (a b) c -> x a (b c)", b=2)
            nc.sync.dma_start(z2v[0], hTe)                       # y even
            hTo = hT[:, 1:63, :].rearrange("x (a b) c -> x a (b c)", b=2)
            nc.sync.dma_start(z2v[1][:, 0:31, :], hTo)           # y odd
            # y = 63: [h(63, x), zeros] (row 64 is outside the grid)
            nc.sync.dma_start(z2v[1][:, 31, 0:128], hT[:, 63, :])
            nc.sync.dma_start(z2v[1][:, 31, 128:256], zrow[:, 0:128])

            # ---- per-block offset conv -> transpose -> masks -> idx fold ----
            # Emitted per gather-block so the first gathers can start while
            # the remaining blocks' offsets are still being computed.
            offT = mpool.tile([96, RPC, 18], F32, tag="offT")
            offP = mpool.tile([128, NT, 18], F32, tag="offP")
            a_w = gpool.tile([128, NT, 9, 4, 2], BF16, tag="a_w")
            idx_w = gpool.tile([32, NBLK, 9, NC16], I16, tag="idx_w")
            obv = ob[:].unsqueeze(1).broadcast_to((128, BLKT, 18))
            TPB = NT // NBLK  # 9 tiles, 12 rows per block
            RPB = RPC // NBLK

            for blk in range(NBLK):
                # offset conv rows of this block (bf16, [w, r, 18])
                for r in range(blk * RPB, (blk + 1) * RPB):
                    gr = r + 8
                    po = psp.tile([96, 18], F32, tag="ps")
                    for t in range(9):
                        ty, tx = t // 3, t % 3
                        lhsT = hn[:, gr + ty - 1, 7 + tx:7 + tx + 96]
                        nc.tensor.matmul(po[:], lhsT, ow[:, t, :],
                                         start=(t == 0), stop=(t == 8))
                    nc.scalar.activation(offT[:, r, :], po[:], AF.Copy)

                # pixel-major transpose of this block's tiles
                g_cur = -1
                pog = None
                for (g, r, w0, p0) in pieces:
                    if not (blk * TPB <= g < (blk + 1) * TPB):
                        continue
                    if g != g_cur:
                        if g_cur >= 0:
                            nc.scalar.activation(offP[:, g_cur, :], pog[:],
                                                 AF.Copy)
                        pog = psp.tile([128, 18], F32, tag="ps")
                        g_cur = g
                    nc.tensor.matmul(pog[p0:p0 + 32, :],
                                     idnf[w0:w0 + 32, w0:w0 + 32],
                                     offT[w0:w0 + 32, r, :],
                                     start=True, stop=True,
                                     tile_position=(w0, p0))
                nc.scalar.activation(offP[:, g_cur, :], pog[:], AF.Copy)
                offPb = offP[:, blk * TPB:(blk + 1) * TPB, :]
                nc.vector.tensor_tensor(offPb, offPb, obv, OP.add)

                # bilinear floor/frac for both y and x in one pass
                def mkb(tag):
                    return mpool.tile([128, TPB, 18], F32, tag=tag, name=tag,
                                      bufs=2)

                it32 = mpool.tile([128, TPB, 18], I32, tag="it32", bufs=2)
                gtb = mkb("gtb")
                fl = mkb("fl")
                fr = mkb("fr")
                uf = mkb("uf")
                nc.vector.tensor_copy(it32[:], offPb)
                nc.vector.tensor_copy(fl[:], it32[:])
                nc.vector.tensor_tensor(gtb[:], fl[:], offPb, OP.is_gt)
                nc.vector.tensor_tensor(fl[:], fl[:], gtb[:], OP.subtract)
                nc.vector.tensor_tensor(fr[:], offPb, fl[:], OP.subtract)
                nc.vector.tensor_scalar(uf[:], fr[:], -1.0, 1.0, OP.mult,
                                        OP.add)
                flv = fl[:].rearrange("p g (k two) -> p g k two", two=2)
                frv = fr[:].rearrange("p g (k two) -> p g k two", two=2)
                ufv = uf[:].rearrange("p g (k two) -> p g k two", two=2)
                fy, fx = flv[:, :, :, 0], flv[:, :, :, 1]
                ly, lx = frv[:, :, :, 0], frv[:, :, :, 1]
                uy, ux = ufv[:, :, :, 0], ufv[:, :, :, 1]

                # corner weights (col-pair order: y0x0, y1x0, y0x1, y1x1),
                # duplicated pairs for bf16 2x vector mode
                def dup2(ap):
                    return ap.unsqueeze(3).broadcast_to((128, TPB, 9, 2))

                a_wb = a_w[:, blk * TPB:(blk + 1) * TPB]
                for j, (fa, fb) in enumerate(((uy, ux), (ly, ux),
                                              (uy, lx), (ly, lx))):
                    nc.vector.tensor_tensor(a_wb[:, :, :, j, :],
                                            dup2(fa), dup2(fb), OP.mult)

                # idx = iota + 112*fy + fx, clamped to [0, NB-2] (keeps both
                # gathered entries in the written region; clamped cases only
                # ever touch zeroed h columns/rows)
                idxf = mpool.tile([128, TPB, 9], F32, tag="idxf", bufs=2)
                nc.vector.tensor_scalar(idxf[:], fy, float(ZX), None, OP.mult)
                nc.vector.tensor_tensor(idxf[:], idxf[:], fx, OP.add)
                nc.vector.tensor_tensor(
                    idxf[:], idxf[:], iot[:, blk * TPB:(blk + 1) * TPB, :],
                    OP.add)
                nc.vector.tensor_scalar(idxf[:], idxf[:], 0.0, float(NB - 2),
                                        OP.max, OP.min)

                # wrapped-16 idx tiles via 8 selection matmuls (128->16 fold):
                # idx_w[p, blk, kk, tb*8+cc] = idx[pixel (blk*9+tb)*128 +
                # cc*16+p]; the queue-0 gather pair only reads partitions 0:32.
                idx_wv = idx_w[0:16, blk].rearrange(
                    "p kk (tb cc) -> p kk tb cc", cc=8)
                rhsx = idxf[:].rearrange("p a b -> p (a b)")
                for cc in range(8):
                    pcc = psp.tile([16, TPB * 9], F32, tag="ps")
                    nc.tensor.matmul(pcc[:], idnf[:, cc * 16:cc * 16 + 16],
                                     rhsx, start=True, stop=True)
                    src = pcc[:].rearrange("p (tb kk) -> p kk tb", kk=9)
                    nc.vector.tensor_copy(idx_wv[:, :, :, cc], src)
                nc.sync.dma_start(idx_w[16:32, blk], idx_w[0:16, blk])

            # overlapping gather source AP: entry stride 256 elems, but each
            # gather reads 512 elems (two adjacent column-pair entries)
            z2ap = z2[:]
            gsrc = AP(z2ap.tensor, z2ap.offset, [[256, NB], [1, 512]])

            # ---- gather + scale + corner-sum/transpose + einsum ----
            d_sb = spool.tile([C, NT, 128], F32, tag="dsb")
            gst = mpool.tile([C, NBLK, 2], F32, tag="gst")
            for blk in range(NBLK):
                sampT = xpool.tile([128, 9, BLKT, 128], BF16, tag="xs")
                for kk in range(9):
                    g_t = gpool.tile([128, BLKT, 4, 128], BF16, tag="g_t",
                                     bufs=2)
                    nc.gpsimd.dma_gather(
                        g_t[:].rearrange("p a b c -> p a (b c)"),
                        gsrc, idx_w[:, blk, kk, :],
                        NIDX, NIDX, 512, elem_step=256, queue_num=0,
                        single_packet=False)
                    # scale corners by bilinear weights (bf16 2x, dup pairs)
                    gv = g_t[:].rearrange("p a b (c two) -> p a b c two",
                                          two=2)
                    for j in range(4):
                        av = a_w[:, blk * BLKT:(blk + 1) * BLKT, kk, j, :]
                        av = av.unsqueeze(2).broadcast_to((128, BLKT, 64, 2))
                        nc.vector.tensor_tensor(gv[:, :, j], gv[:, :, j], av,
                                                OP.mult)
                    # sum 4 corners + transpose to channel-major via PE
                    for t in range(BLKT):
                        pt = psp.tile([128, 128], F32, tag="ps")
                        for j in range(4):
                            nc.tensor.matmul(pt[:], g_t[:, t, j, :], idn[:],
                                             start=(j == 0), stop=(j == 3))
                        nc.scalar.activation(sampT[:, kk, t, :], pt[:],
                                             AF.Copy)
                # einsum: accumulate 9 taps
                for t in range(BLKT):
                    pd = psp.tile([128, 128], F32, tag="ps")
                    for kk in range(9):
                        nc.tensor.matmul(pd[:], wd[:, kk, :],
                                         sampT[:, kk, t, :],
                                         start=(kk == 0), stop=(kk == 8))
                    nc.scalar.activation(d_sb[:, blk * BLKT + t, :], pd[:],
                                         AF.Identity, bias=db[:])
                # per-block GroupNorm partial stats (overlaps the next block)
                dblk = d_sb[:, blk * BLKT:(blk + 1) * BLKT, :]
                nc.vector.tensor_reduce(gst[:, blk, 0:1], dblk,
                                        mybir.AxisListType.XY, OP.add)
                nc.scalar.activation(
                    sq[:, blk * BLKT * 128:(blk + 1) * BLKT * 128].rearrange(
                        "p (a b) -> p a b", a=BLKT),
                    dblk, AF.Square,
                    accum_out=gst[:, blk, 1:2])

            # ---- GroupNorm stats (whole sample) ----
            # cross-channel sum replicated to all partitions via an all-ones
            # matmul, so no partition_broadcast is needed after the AllReduce
            gss = mpool.tile([C, 2], F32, tag="gss")
            nc.vector.tensor_tensor(gss[:], gst[:, 0, :], gst[:, 1, :],
                                    OP.add)
            nc.vector.tensor_tensor(gss[:], gss[:], gst[:, 2, :], OP.add)
            nc.vector.tensor_tensor(gss[:], gss[:], gst[:, 3, :], OP.add)
            pg = psp.tile([128, 2], F32, tag="ps")
            nc.tensor.matmul(pg[:], onesf[:], gss[:], start=True, stop=True)
            gred = mpool.tile([128, 2], F32, tag="gred")
            nc.scalar.activation(gred[:], pg[:], AF.Copy)
            ccg_in = dramp.tile([128, 2], F32, tag="ccgi")
            ccg_out = dramp.tile([128, 2], F32, tag="ccgo")
            nc.sync.dma_start(ccg_in[:], gred[:])
            nc.gpsimd.collective_compute(
                "AllReduce", OP.add, replica_groups=groups,
                ins=[ccg_in[:].opt()], outs=[ccg_out[:].opt()])
            gall = mpool.tile([128, 2], F32, tag="gall")
            nc.sync.dma_start(gall[:], ccg_out[:])

            gmom = mpool.tile([C, 2], F32, tag="gmom")
            nc.vector.tensor_scalar(gmom[:], gall[:], 1.0 / (C * H * W), None,
                                    OP.mult)
            gvar = mpool.tile([C, 1], F32, tag="gvar")
            nc.vector.tensor_tensor(gvar[:], gmom[:, 0:1], gmom[:, 0:1],
                                    OP.mult)
            nc.vector.tensor_tensor(gvar[:], gmom[:, 1:2], gvar[:],
                                    OP.subtract)
            nc.vector.tensor_scalar(gvar[:], gvar[:], EPS, None, OP.add)
            grstd = mpool.tile([C, 1], F32, tag="grstd")
            nc.scalar.activation(grstd[:], gvar[:], AF.Sqrt)
            nc.vector.reciprocal(grstd[:], grstd[:])
            # scale2 = gn_w * rstd ; bias2 = gn_b - mean * scale2
            sc2 = mpool.tile([C, 1], F32, tag="sc2")
            nc.vector.tensor_tensor(sc2[:], gw[:], grstd[:], OP.mult)
            bi2 = mpool.tile([C, 1], F32, tag="bi2")
            nc.vector.tensor_tensor(bi2[:], gmom[:, 0:1], sc2[:], OP.mult)
            nc.vector.tensor_tensor(bi2[:], gb[:], bi2[:], OP.subtract)

            # ---- gate + residual (two halves so the first output DMA
            # overlaps the second half's gate math) ----
            gg = spool.tile([C, NPIX], F32, tag="big")  # reuse sq slot
            HT2 = NT // 2
            HP = HT2 * 128
            HR = RPC // 2
            for hh in range(2):
                ggh = gg[:, hh * HP:(hh + 1) * HP]
                nc.scalar.activation(
                    ggh.rearrange("p (a b) -> p a b", a=HT2),
                    d_sb[:, hh * HT2:(hh + 1) * HT2, :], AF.Sigmoid,
                    bias=bi2[:], scale=sc2[:])
                nc.vector.tensor_scalar(ggh, ggh, 1.0, None, OP.add)
                nc.vector.tensor_tensor(ggh, ggh,
                                        short[:, hh * HP:(hh + 1) * HP],
                                        OP.mult)
                nc.sync.dma_start(
                    out_d[:, hh * HR:(hh + 1) * HR, :],
                    ggh.rearrange("p (r w) -> p r w", w=W))

    nc.compile()
    return nc


def _prep_inputs(inputs):
    x = np.asarray(inputs["x"], np.float32)
    dw_w = np.asarray(inputs["dw_w"], np.float32)
    dw_b = np.asarray(inputs["dw_b"], np.float32)
    pw_w = np.asarray(inputs["pw_w"], np.float32)
    pw_b = np.asarray(inputs["pw_b"], np.float32)
    off_w = np.asarray(inputs["off_w"], np.float32)
    off_b = np.asarray(inputs["off_b"], np.float32)
    de_w = np.asarray(inputs["de_w"], np.float32)
    de_b = np.asarray(inputs["de_b"], np.float32)
    gn_w = np.asarray(inputs["gn_w"], np.float32)
    gn_b = np.asarray(inputs["gn_b"], np.float32)

    bf = ml_dtypes.bfloat16
    # fused conv1 weights: wf[t][c, o] = pw_w[o, c] * dw_w[c, 0, ty, tx]
    dwt = dw_w.reshape(C, 9)                        # [c, t]
    wf = np.ascontiguousarray(
        (pw_w.T[None, :, :] * dwt.T[:, :, None]).transpose(1, 0, 2)
    ).astype(np.float32)                            # [c, t, o]
    b1 = (pw_w @ dw_b + pw_b).astype(np.float32).reshape(C, 1)
    ow = np.ascontiguousarray(
        off_w.reshape(18, C, 9).transpose(1, 2, 0)).astype(bf)   # [c, t, 18]
    obr = np.broadcast_to(off_b[None, :], (128, 18)).astype(np.float32)
    obr = np.ascontiguousarray(obr)
    wdm = np.ascontiguousarray(
        de_w.reshape(C, C, 9).transpose(1, 2, 0)).astype(bf)     # [c, k, o]
    dbc = de_b.reshape(C, 1).astype(np.float32)
    gwc = gn_w.reshape(C, 1).astype(np.float32)
    gbc = gn_b.reshape(C, 1).astype(np.float32)
    idn = np.eye(128, dtype=bf)
    idnf = np.eye(128, dtype=np.float32)
    onec = np.ones((C, 1), np.float32)
    onesf = np.ones((C, 128), np.float32)
    # pixel-major iota: pixel = g*128 + p, (r, w) = divmod(pixel, 96);
    # base entry = (8 + r + ky)*112 + (8 + w + kx)
    pixv = np.arange(NPIX)
    rv = (pixv // 96).reshape(NT, 128)
    wv = (pixv % 96).reshape(NT, 128)
    kyv = (np.arange(9) // 3 - 1)
    kxv = (np.arange(9) % 3 - 1)
    iotc = (ZX * (8 + rv[:, :, None] + kyv[None, None, :])
            + 8 + wv[:, :, None] + kxv[None, None, :])
    iotc = np.ascontiguousarray(
        iotc.transpose(1, 0, 2)).astype(np.float32)  # [128, NT, 9]

    in_maps = []
    for core in range(NCORES):
        b = core // 2
        r0 = (core % 2) * RPC
        xp = np.zeros((C, XR, XC), np.float32)
        glo, ghi = max(0, r0 - 9), min(H, r0 + 57)
        xp[:, glo - (r0 - 9):ghi - (r0 - 9), 9:105] = x[b, :, glo:ghi, :]
        vrow = np.zeros((C, GR), bf)
        vlo, vhi = max(0, r0 - 8), min(H, r0 + 56)
        vrow[:, vlo - (r0 - 8):vhi - (r0 - 8)] = bf(1.0)
        in_maps.append({
            "x_sh": xp, "vrow": vrow, "wf": wf, "b1c": b1, "ow": ow,
            "obr": obr, "wd": wdm, "dbc": dbc, "gwc": gwc, "gbc": gbc,
            "idn": idn, "idnf": idnf, "onec": onec, "onesf": onesf,
            "iotc": iotc,
        })
    return in_maps


def get_program():
    if "nc" not in _CACHE:
        _CACHE["nc"] = _build_program()
    return _CACHE["nc"]


def kernel(**inputs):
    nc = get_program()
    in_maps = _prep_inputs(inputs)
    res = run_bass_kernel_spmd(nc, in_maps, core_ids=list(range(NCORES)))
    out = np.empty((B, C, H, W), np.float32)
    for core in range(NCORES):
        b = core // 2
        r0 = (core % 2) * RPC
        out[b, :, r0:r0 + RPC, :] = res.results[core]["out_sh"]
    return out
